# revision 2
# baseline (speedup 1.0000x reference)
"""GCN layer (GCNConv forward) on 8 Trainium2 NeuronCores — v2.

out = D^-1/2 (A+I) D^-1/2 (x @ W) + b   with random edge_index [2, E].

Strategy (dest-sharded, as v1) with a reworked gather path:
  - y = dinv * (x@W) stored in HBM as ONE bf16 table [N_PAD, C]
  - per-edge messages fetched as 256B PAIR elements (2 consecutive bf16 rows)
    through two views of the table: even-src edges use row offset 0, odd-src
    edges use row offset 1; idx = src>>1 in both cases, so the gathered
    slice [:, slot, 0:64] is exactly y[src] in bf16 (no convert, no select)
  - edge slots grouped by (bank, parity, dest-tile); 2 banks of 32768 pairs
    keep idx within int16 range
  - dma_gather calls round-robin over 4 SWDGE queues (desc-gen runs on a
    different Q7 core pair per queue -> 4x parallel descriptor generation)
  - indicator matrices for a whole 1024-edge call built in ONE DVE op
    (iota vs broadcast colrel, batched is_equal)
  - phase 1 scales 8 matmul outputs per window with one broadcast DVE mul
"""
import os
import sys

if "/opt/trn_rl_repo" not in sys.path:
    sys.path.insert(0, "/opt/trn_rl_repo")

import numpy as np
import ml_dtypes
from contextlib import ExitStack

import concourse.bacc as bacc
import concourse.bass as bass
import concourse.mybir as mybir
import concourse.tile as tile
from concourse import library_config
from concourse._compat import cdiv
from concourse.bass_utils import run_bass_kernel_spmd

# ---------------- problem constants (hardcoded per spec) ----------------
N = 100000
E = 1600000
C = 64
NCORES = 8
NSHARD = N // NCORES            # 12500 dest rows per core
P = 128
NT = cdiv(NSHARD, P)            # 98 dest tiles per core (12544 padded)
PAIR_BANK = 32768               # pairs per bank (int16 idx range)
NPAIR = 50176                   # N_PAD // 2
NBANK = cdiv(NPAIR, PAIR_BANK)  # 2
NSB = 2 * NBANK                 # (bank, parity) superblocks: sb = bank*2+parity
CALL = int(os.environ.get("GCN_CALL", "1024"))  # edges per dma_gather call
NQ = int(os.environ.get("GCN_NQ", "4"))         # SWDGE queues (1..4)
WIN = 1024                      # xw phase: nodes per y-write window (wrap-8)
WRAP = 8                        # consecutive y rows per partition in a window
XT_BLOCK = 12288                # nodes per xT SBUF block (2 halves of 6144)
N_PAD = 100352                  # 8*12288 + 2048; multiple of 1024
NU = N_PAD // P                 # 784 dinv columns
PADCOL = 200.0                  # pad colrel value (exact in bf16, never matches)

BF16 = ml_dtypes.bfloat16


def _wrap_node_index():
    """node id at (p, u) of the wrap-WRAP dinv layout."""
    p = np.arange(P)[:, None]
    u = np.arange(NU)[None, :]
    return (u // WRAP) * WIN + p * WRAP + (u % WRAP)


# ---------------- host-side preprocessing ----------------
def preprocess(x, edge_index, W, b):
    x = np.asarray(x, np.float32)
    edge_index = np.asarray(edge_index)
    W = np.asarray(W, np.float32)
    b = np.asarray(b, np.float32)
    row = edge_index[0].astype(np.int64)
    col = edge_index[1].astype(np.int64)

    cnt = np.bincount(col, minlength=N).astype(np.int64)
    rowptr = np.concatenate([[0], np.cumsum(cnt)])

    # append self-loops (message y[n] -> dest n), then shard by dest
    loops = np.arange(N, dtype=np.int64)
    row = np.concatenate([row, loops])
    col = np.concatenate([col, loops])

    shard = col // NSHARD
    NG = NSB * NT                        # (superblock, tile) groups
    per_core = []
    counts = np.zeros((NCORES, NG), np.int64)
    for c in range(NCORES):
        m = shard == c
        r = row[m]
        cl = col[m] - c * NSHARD
        pair = r >> 1
        sb = (pair // PAIR_BANK) * 2 + (r & 1)
        g = sb * NT + cl // P
        order = np.argsort(g, kind="stable")
        r, cl, g = r[order], cl[order], g[order]
        counts[c] = np.bincount(g, minlength=NG)
        per_core.append((r, cl, g))

    # 16-granular quotas (idx stream granularity); chunks may then straddle
    # two groups, handled by a second indicator stream on boundary chunks.
    # Each superblock stream is padded to a 128 multiple via its last group.
    quota = (np.ceil(counts.max(axis=0) / 16).astype(np.int64)) * 16   # [NG]
    for s in range(NSB):
        sblen = int(quota[s * NT:(s + 1) * NT].sum())
        quota[(s + 1) * NT - 1] += (-sblen) % P
    qoff = np.concatenate([[0], np.cumsum(quota)])
    total = int(qoff[-1])

    sb_len = [int(quota[s * NT:(s + 1) * NT].sum()) for s in range(NSB)]
    sb_off = np.concatenate([[0], np.cumsum(sb_len)]).astype(np.int64)
    calls = []                            # (sb, stream_start, n_idx)
    for s in range(NSB):
        st = int(sb_off[s])
        while st < int(sb_off[s + 1]):
            n = min(CALL, int(sb_off[s + 1]) - st)
            calls.append((s, st, n))
            st += n

    struct = {"quota": quota.tolist(), "qoff": qoff.tolist(), "total": total,
              "calls": calls}

    # ---- shared arrays ----
    S16, S128 = total // 16, total // 128
    xT = np.zeros((C, N_PAD), np.float32)
    xT[:, :N] = x.T
    xT = np.ascontiguousarray(xT.astype(BF16))
    W_bf = np.ascontiguousarray(np.tile(W, (2, 1)).astype(BF16))  # both halves
    b_bcast = np.ascontiguousarray(np.tile(b[None, :], (P, 1)).astype(np.float32))

    nid = _wrap_node_index()
    valid = nid < N
    rpA = np.zeros((P, NU), np.float32)
    rpB = np.zeros((P, NU), np.float32)
    rpA[valid] = rowptr[nid[valid]]
    rpB[valid] = rowptr[nid[valid] + 1]

    in_maps = []
    for c in range(NCORES):
        r, cl, g = per_core[c]
        cnt_c = counts[c]
        gstart = np.concatenate([[0], np.cumsum(cnt_c)])
        rank = np.arange(len(g)) - gstart[g]
        pos = qoff[g] + rank

        # pad slots must gather *something*; spreading their indices across
        # the bank avoids hammering one 256B HBM row from ~18% of descriptors
        npb = [PAIR_BANK, NPAIR - PAIR_BANK]
        idx_rel = np.concatenate(
            [np.arange(quota[gg]) % npb[gg // NT // 2] for gg in range(NG)])
        colrel = np.full(total, PADCOL, np.float32)     # pads never match iota
        idx_rel[pos] = (r >> 1) - (g // NT // 2) * PAIR_BANK
        colrel[pos] = cl - (g % NT) * P

        # dual indicator streams: slots whose group owns the chunk start go
        # to stream A (fused per-call build); a chunk's second group goes to
        # stream B (small per-boundary-chunk build)
        slot_group = np.repeat(np.arange(NG), quota)
        first_of_chunk = np.repeat(slot_group[::P][:, None], P, 1).ravel()
        a_mask = slot_group == first_of_chunk
        assert np.all((slot_group - first_of_chunk) <= 1), "chunk spans >2 groups"
        colrelA = np.where(a_mask, colrel, PADCOL)
        colrelB = np.where(a_mask, PADCOL, colrel)

        idx16 = np.zeros((16, S16), np.int16)
        idx16[np.arange(total) % 16, np.arange(total) // 16] = idx_rel
        idx16 = np.ascontiguousarray(np.tile(idx16, (8, 1)))

        def pack_colr(cr):
            cc = np.zeros((P, S128), np.float32)
            cc[np.arange(total) % P, np.arange(total) // P] = cr
            return np.ascontiguousarray(cc.astype(BF16))
        colr = pack_colr(colrelA)
        colrB = pack_colr(colrelB)

        pp = np.arange(P)[:, None]
        tt = np.arange(NT)[None, :]
        nd = c * NSHARD + tt * P + pp
        vd = nd < N
        rpdA = np.zeros((P, NT), np.float32)
        rpdB = np.zeros((P, NT), np.float32)
        rpdA[vd] = rowptr[nd[vd]]
        rpdB[vd] = rowptr[nd[vd] + 1]

        in_maps.append({
            "xT": xT, "W": W_bf, "bb": b_bcast, "rpA": rpA, "rpB": rpB,
            "rpdA": np.ascontiguousarray(rpdA),
            "rpdB": np.ascontiguousarray(rpdB),
            "idx16": idx16, "colrel": colr, "colrelB": colrB,
        })
    return in_maps, struct


# ---------------- device program ----------------
def build_program(struct):
    quota = struct["quota"]
    qoff = struct["qoff"]
    total = struct["total"]
    all_calls = struct["calls"]
    S16, S128 = total // 16, total // 128
    phases = os.environ.get("GCN_PHASES", "123")
    skip = os.environ.get("GCN_SKIP", "")
    rep = int(os.environ.get("GCN_REPEAT", "1"))
    maxcalls = int(os.environ.get("GCN_MAXCALLS", "1000000"))

    nc = bacc.Bacc("TRN2", target_bir_lowering=False, debug=True,
                   dynamic_dma_scratch_size=16 * CALL,
                   num_swdge_queues=NQ)
    f32, bf16, i16 = mybir.dt.float32, mybir.dt.bfloat16, mybir.dt.int16

    xT_d = nc.dram_tensor("xT", [C, N_PAD], bf16, kind="ExternalInput")
    W_d = nc.dram_tensor("W", [2 * C, C], bf16, kind="ExternalInput")
    bb_d = nc.dram_tensor("bb", [P, C], f32, kind="ExternalInput")
    rpA_d = nc.dram_tensor("rpA", [P, NU], f32, kind="ExternalInput")
    rpB_d = nc.dram_tensor("rpB", [P, NU], f32, kind="ExternalInput")
    rpdA_d = nc.dram_tensor("rpdA", [P, NT], f32, kind="ExternalInput")
    rpdB_d = nc.dram_tensor("rpdB", [P, NT], f32, kind="ExternalInput")
    idx_d = nc.dram_tensor("idx16", [P, S16], i16, kind="ExternalInput")
    colr_d = nc.dram_tensor("colrel", [P, S128], bf16, kind="ExternalInput")
    colrB_d = nc.dram_tensor("colrelB", [P, S128], bf16, kind="ExternalInput")
    out_d = nc.dram_tensor("out", [P, NT, C], f32, kind="ExternalOutput")
    # y table split at the bank boundary so bank-0 gathers can start while
    # phase 1 is still producing bank-1 rows. y0 = rows [0, 65536] (the odd
    # view of pair 32767 needs row 65536), y1 = rows [65536, N_PAD+2).
    Y0_ROWS = 2 * PAIR_BANK + 2
    Y1_ROWS = N_PAD + 2 - 2 * PAIR_BANK
    y0_d = nc.dram_tensor("ytab0", [Y0_ROWS, C], bf16, kind="Internal")
    y1_d = nc.dram_tensor("ytab1", [Y1_ROWS, C], bf16, kind="Internal")

    with tile.TileContext(nc) as tc:
        with ExitStack() as ctx:
            const = ctx.enter_context(tc.tile_pool(name="const", bufs=1))
            psum_pool = ctx.enter_context(
                tc.tile_pool(name="psum", bufs=4, space="PSUM"))
            psum_mm = ctx.enter_context(
                tc.tile_pool(name="psummm", bufs=4, space="PSUM"))
            dtmp = ctx.enter_context(tc.tile_pool(name="dtmp", bufs=1))
            xtp = ctx.enter_context(tc.tile_pool(name="xt", bufs=2))
            ysbp = ctx.enter_context(tc.tile_pool(name="ysb", bufs=3))
            dvp = ctx.enter_context(tc.tile_pool(name="dvp", bufs=4))
            gbp = ctx.enter_context(tc.tile_pool(name="gb", bufs=6))
            indp = ctx.enter_context(tc.tile_pool(name="ind", bufs=4))
            crp = ctx.enter_context(tc.tile_pool(name="crep", bufs=4))
            indBp = ctx.enter_context(tc.tile_pool(name="indB", bufs=4))
            crpB = ctx.enter_context(tc.tile_pool(name="crB", bufs=4))

            nc.gpsimd.load_library(library_config.mlp)

            W_sb = const.tile([2 * C, C], bf16, tag="W")
            bb_sb = const.tile([P, C], f32, tag="bb")
            iota_i = const.tile([P, P], i16, tag="iota_i")
            iota8 = const.tile([P, CALL // P, P], bf16, tag="iota8")
            dinv_g = const.tile([P, NU], f32, tag="dinv_g")
            dinv_d = const.tile([P, NT], f32, tag="dinv_d")
            acc = const.tile([P, NT * C], f32, tag="acc")
            idx_sb = const.tile([P, S16], i16, tag="idx")
            colr_sb = const.tile([P, S128], bf16, tag="colr")
            colrB_sb = const.tile([P, S128], bf16, tag="colrB")

            nc.sync.dma_start(W_sb[:], W_d[:])
            nc.sync.dma_start(bb_sb[:], bb_d[:])
            nc.sync.dma_start(idx_sb[:], idx_d[:])
            nc.sync.dma_start(colr_sb[:], colr_d[:])
            nc.sync.dma_start(colrB_sb[:], colrB_d[:])
            nc.gpsimd.iota(iota_i[:], pattern=[[1, P]], channel_multiplier=0)
            nc.vector.memset(acc[:], 0.0)
            for j in range(CALL // P):
                nc.vector.tensor_copy(iota8[:, j, :], iota_i[:])
            # zero the tail rows of each y table that fall inside a gather
            # view's declared region but are never written by phase 1
            ztail = const.tile([2, C], bf16, tag="ztail")
            nc.vector.memset(ztail[:], 0.0)
            nc.sync.dma_start(
                bass.AP(y1_d, (Y1_ROWS - 2) * C, [[C, 2], [1, C]]), ztail[:])

            def emit_body():
                # ---- dinv = sqrt(1 / (rowptr[n+1]-rowptr[n]+1)) ----
                for (ad, bd, w, dst) in ((rpA_d, rpB_d, NU, dinv_g),
                                         (rpdA_d, rpdB_d, NT, dinv_d)):
                    ta = dtmp.tile([P, NU], f32, tag="ta", name="ta")
                    tb = dtmp.tile([P, NU], f32, tag="tb", name="tb")
                    nc.sync.dma_start(ta[:, :w], ad[:])
                    nc.sync.dma_start(tb[:, :w], bd[:])
                    nc.vector.tensor_tensor(tb[:, :w], tb[:, :w], ta[:, :w],
                                            mybir.AluOpType.subtract)
                    nc.vector.tensor_scalar_add(tb[:, :w], tb[:, :w], 1.0)
                    nc.vector.reciprocal(ta[:, :w], tb[:, :w])
                    nc.scalar.activation(dst[:], ta[:, :w],
                                         mybir.ActivationFunctionType.Sqrt)

                # ---- phase 1: y = dinv * (x @ W), bf16 table ----
                blocks = []
                base = 0
                while base < N_PAD and "1" in phases:
                    nblk = min(XT_BLOCK, N_PAD - base)
                    blocks.append((base, nblk))
                    base += nblk
                GRP = 4          # windows per ytab write DMA (4096 rows)
                for (base, nblk) in blocks:
                    half = nblk // 2
                    xt = xtp.tile([P, XT_BLOCK // 2], bf16, tag="xt", name="xt")
                    src = bass.AP(xT_d, base,
                                  [[half, 2], [N_PAD, C], [1, half]])
                    nc.sync.dma_start(xt[:, :half], src)
                    nwin = nblk // WIN
                    for wg in range(cdiv(nwin, GRP)):
                        gcnt = min(GRP, nwin - wg * GRP)
                        gbase = base + wg * GRP * WIN
                        ysb = ysbp.tile([P, GRP, WRAP, C], bf16, tag="ysb",
                                        name="ysb")
                        for wi in range(gcnt):
                            w = wg * GRP + wi
                            wbase = base + w * WIN
                            h = (w * WIN) // half
                            foff = (w * WIN) % half
                            ps = psum_pool.tile([P, WRAP * C], f32, tag="mm",
                                                name="mmps")
                            u0 = (wbase // WIN) * WRAP
                            for s in range(WRAP):
                                lhsT = xt[h * C:(h + 1) * C,
                                          foff + s:
                                          foff + s + WRAP * (P - 1) + 1: WRAP]
                                nc.tensor.matmul(ps[:, s * C:(s + 1) * C],
                                                 lhsT,
                                                 W_sb[h * C:(h + 1) * C, :],
                                                 start=True, stop=True)
                            dv = dinv_g[:, u0:u0 + WRAP].unsqueeze(2)
                            dvrep = dvp.tile([P, WRAP, C], f32, tag="dvrep",
                                             name="dvrep")
                            nc.scalar.activation(
                                dvrep[:], dv.broadcast_to((P, WRAP, C)),
                                mybir.ActivationFunctionType.Copy)
                            nc.vector.tensor_tensor(
                                ysb[:, wi, :, :],
                                ps[:].rearrange("p (s c) -> p s c", c=C),
                                dvrep[:], mybir.AluOpType.mult)
                        ap4 = [[WRAP * C, P], [WIN * C, gcnt], [C, WRAP],
                               [1, C]]
                        if gbase + gcnt * WIN <= 2 * PAIR_BANK:
                            dst = bass.AP(y0_d, gbase * C, ap4)
                        else:
                            dst = bass.AP(y1_d, (gbase - 2 * PAIR_BANK) * C,
                                          ap4)
                        # ACT's HWDGE ring: parallel to SP's xt loads
                        nc.scalar.dma_start(dst, ysb[:, :gcnt, :, :])
                        if gbase == 2 * PAIR_BANK:
                            # row 65536 = (p=0, wi=0, s=0) of this group also
                            # belongs to y0 (odd view of pair 32767)
                            nc.scalar.dma_start(
                                bass.AP(y0_d, 2 * PAIR_BANK * C, [[C, 1],
                                                                  [1, C]]),
                                ysb[0:1, 0, 0, :])

                # ---- phase 2: pair-gather + indicator matmuls ----
                calls = all_calls if "2" in phases else []
                calls = calls[:maxcalls]
                grp_first_sb = [None] * NT
                grp_last_sb = [None] * NT
                for t in range(NT):
                    for s in range(NSB):
                        if quota[s * NT + t] > 0:
                            if grp_first_sb[t] is None:
                                grp_first_sb[t] = s
                            grp_last_sb[t] = s

                npair_bank = [PAIR_BANK, NPAIR - PAIR_BANK]
                ytabs = [y0_d, y1_d]
                psum_by_tile = {}
                for ci, (sb, cstart, cn) in enumerate(calls):
                    bk, par = sb // 2, sb % 2
                    nslots = cn // P
                    gbuf = gbp.tile([P, CALL // P, 2 * C], bf16, tag="gbuf",
                                    name="gbuf")
                    view = bass.AP(ytabs[bk], par * C,
                                   [[2 * C, npair_bank[bk]], [1, 2 * C]])
                    if "g" not in skip:
                        nc.gpsimd.dma_gather(
                            gbuf[:, :nslots, :], view,
                            idx_sb[:, cstart // 16: (cstart + cn) // 16],
                            cn, cn, 2 * C, queue_num=ci % NQ)
                    else:
                        nc.vector.memset(gbuf[:, :nslots, :], 0.5)

                    if "m" in skip:     # pure-gather ablation
                        continue
                    # colrep materialized on ACT (idle engine; never contends
                    # with GPSIMD SBUF ports), then a two-stream is_equal on
                    # DVE (tensor_tensor stays in 1-port mode)
                    ind = indp.tile([P, CALL // P, P], bf16, tag="ind",
                                    name="ind")
                    crep = crp.tile([P, CALL // P, P], bf16, tag="crep",
                                    name="crep")
                    if "i" not in skip:
                        cb = colr_sb[:, cstart // P: cstart // P + nslots]
                        nc.scalar.activation(
                            crep[:, :nslots, :],
                            cb.unsqueeze(2).broadcast_to((P, nslots, P)),
                            mybir.ActivationFunctionType.Copy)
                        nc.vector.tensor_tensor(
                            ind[:, :nslots, :], iota8[:, :nslots, :],
                            crep[:, :nslots, :], mybir.AluOpType.is_equal)
                    else:
                        nc.scalar.activation(ind[:, :nslots, :],
                                             iota8[:, :nslots, :],
                                             mybir.ActivationFunctionType.Copy)

                    # second-group indicators for chunks straddling a group
                    # boundary (16-granular quotas)
                    indB_tiles = {}
                    if "i" not in skip:
                        for t in range(NT):
                            g = sb * NT + t
                            g0 = qoff[g]
                            if quota[g] == 0 or g0 % P == 0:
                                continue
                            if not (cstart <= (g0 // P) * P < cstart + cn):
                                continue
                            ch = g0 // P
                            crB = crpB.tile([P, P], bf16, tag="crB",
                                            name="crB")
                            bt = indBp.tile([P, P], bf16, tag="indB",
                                            name="indB")
                            nc.scalar.activation(
                                crB[:],
                                colrB_sb[:, ch:ch + 1].broadcast_to((P, P)),
                                mybir.ActivationFunctionType.Copy)
                            nc.vector.tensor_tensor(
                                bt[:], iota8[:, 0, :], crB[:],
                                mybir.AluOpType.is_equal)
                            indB_tiles[ch] = bt

                    for t in range(NT):
                        g = sb * NT + t
                        q = quota[g]
                        if q == 0:
                            continue
                        g0, g1 = qoff[g], qoff[g] + q
                        lo, hi = max(g0, cstart), min(g1, cstart + cn)
                        if lo >= hi:
                            continue
                        if lo == g0:
                            psum_by_tile[t] = psum_mm.tile(
                                [P, C], f32, tag="mm2", name=f"pst_s{sb}_t{t}")
                        ps = psum_by_tile[t]
                        first_cg, last_cg = g0 // P, (g1 - 1) // P
                        for chunk in range(lo // P, (hi + P - 1) // P):
                            slot = chunk - cstart // P
                            if g0 > chunk * P and chunk in indB_tiles:
                                lhs = indB_tiles[chunk][:]
                            else:
                                lhs = ind[:, slot, :]
                            nc.tensor.matmul(
                                ps[:], lhs, gbuf[:, slot, :C],
                                start=(chunk == first_cg),
                                stop=(chunk == last_cg))
                        if hi == g1:
                            a = acc[:, t * C:(t + 1) * C]
                            if sb == grp_first_sb[t]:
                                # ACT copy: a DVE tensor_copy here would grab
                                # the shared SBUF port pair and stall SWDGE
                                # descriptor generation mid-gather
                                nc.scalar.activation(
                                    a, ps[:],
                                    mybir.ActivationFunctionType.Copy)
                            else:
                                nc.vector.tensor_tensor(
                                    a, a, ps[:], mybir.AluOpType.add)
                            if sb == grp_last_sb[t] and "3" in phases:
                                # fold phase 3 into the final close so the
                                # tail after the last gather stays short;
                                # ACT scale + DVE add never contend with
                                # SWDGE descriptor generation
                                nc.scalar.activation(
                                    a, a, mybir.ActivationFunctionType.Copy,
                                    scale=dinv_d[:, t: t + 1])
                                nc.vector.tensor_tensor(
                                    a, a, bb_sb[:], mybir.AluOpType.add)
                            del psum_by_tile[t]

                for t, ps in list(psum_by_tile.items()):
                    # truncated-call debug runs leave open groups; close them
                    nc.scalar.activation(acc[:, t * C:(t + 1) * C], ps[:],
                                         mybir.ActivationFunctionType.Copy)
                    del psum_by_tile[t]

                # ---- phase 3 is folded into each tile's final close ----
                if "2" not in phases and "3" in phases:
                    for t in range(NT):
                        a = acc[:, t * C:(t + 1) * C]
                        nc.vector.tensor_scalar_mul(a, a, dinv_d[:, t: t + 1])
                        nc.vector.tensor_tensor(a, a, bb_sb[:],
                                                mybir.AluOpType.add)
                nc.sync.dma_start(
                    out_d[:], acc[:].rearrange("p (t c) -> p t c", c=C))

            if rep > 1:
                with tc.For_i(0, rep, 1):
                    emit_body()
            else:
                emit_body()

    nc.compile()
    return nc


# ---------------- entry point ----------------
_CACHE = {}


def kernel(x, edge_index, W, b):
    in_maps, struct = preprocess(x, edge_index, W, b)
    key = (struct["total"], tuple(struct["quota"]))
    if key not in _CACHE:
        _CACHE.clear()
        _CACHE[key] = build_program(struct)
    nc = _CACHE[key]
    res = run_bass_kernel_spmd(nc, in_maps, core_ids=list(range(NCORES)))
    outs = []
    for c in range(NCORES):
        o = res.results[c]["out"]                      # [P, NT, C]
        o = np.transpose(o, (1, 0, 2)).reshape(NT * P, C)[:NSHARD]
        outs.append(o)
    return np.concatenate(outs, axis=0).astype(np.float32)


# revision 3
# speedup vs baseline: 1.0055x; 1.0055x over previous
"""GCN layer (GCNConv forward) on 8 Trainium2 NeuronCores — v2.

out = D^-1/2 (A+I) D^-1/2 (x @ W) + b   with random edge_index [2, E].

Strategy (dest-sharded, as v1) with a reworked gather path:
  - y = dinv * (x@W) stored in HBM as ONE bf16 table [N_PAD, C]
  - per-edge messages fetched as 256B PAIR elements (2 consecutive bf16 rows)
    through two views of the table: even-src edges use row offset 0, odd-src
    edges use row offset 1; idx = src>>1 in both cases, so the gathered
    slice [:, slot, 0:64] is exactly y[src] in bf16 (no convert, no select)
  - edge slots grouped by (bank, parity, dest-tile); 2 banks of 32768 pairs
    keep idx within int16 range
  - dma_gather calls round-robin over 4 SWDGE queues (desc-gen runs on a
    different Q7 core pair per queue -> 4x parallel descriptor generation)
  - indicator matrices for a whole 1024-edge call built in ONE DVE op
    (iota vs broadcast colrel, batched is_equal)
  - phase 1 scales 8 matmul outputs per window with one broadcast DVE mul
"""
import os
import sys

if "/opt/trn_rl_repo" not in sys.path:
    sys.path.insert(0, "/opt/trn_rl_repo")

import numpy as np
import ml_dtypes
from contextlib import ExitStack

import concourse.bacc as bacc
import concourse.bass as bass
import concourse.mybir as mybir
import concourse.tile as tile
from concourse import library_config
from concourse._compat import cdiv
from concourse.bass_utils import run_bass_kernel_spmd

# ---------------- problem constants (hardcoded per spec) ----------------
N = 100000
E = 1600000
C = 64
NCORES = 8
NSHARD = N // NCORES            # 12500 dest rows per core
P = 128
NT = cdiv(NSHARD, P)            # 98 dest tiles per core (12544 padded)
PAIR_BANK = 32768               # pairs per bank (int16 idx range)
NPAIR = 50176                   # N_PAD // 2
NBANK = cdiv(NPAIR, PAIR_BANK)  # 2
NSB = 2 * NBANK                 # (bank, parity) superblocks: sb = bank*2+parity
CALL = int(os.environ.get("GCN_CALL", "1024"))  # edges per dma_gather call
NQ = int(os.environ.get("GCN_NQ", "4"))         # SWDGE queues (1..4)
WIN = 1024                      # xw phase: nodes per y-write window (wrap-8)
WRAP = 8                        # consecutive y rows per partition in a window
XT_BLOCK = 12288                # nodes per xT SBUF block (2 halves of 6144)
N_PAD = 100352                  # 8*12288 + 2048; multiple of 1024
NU = N_PAD // P                 # 784 dinv columns
PADCOL = 200.0                  # pad colrel value (exact in bf16, never matches)

BF16 = ml_dtypes.bfloat16


def _wrap_node_index():
    """node id at (p, u) of the wrap-WRAP dinv layout."""
    p = np.arange(P)[:, None]
    u = np.arange(NU)[None, :]
    return (u // WRAP) * WIN + p * WRAP + (u % WRAP)


# ---------------- host-side preprocessing ----------------
def preprocess(x, edge_index, W, b):
    x = np.asarray(x, np.float32)
    edge_index = np.asarray(edge_index)
    W = np.asarray(W, np.float32)
    b = np.asarray(b, np.float32)
    row = edge_index[0].astype(np.int64)
    col = edge_index[1].astype(np.int64)

    cnt = np.bincount(col, minlength=N).astype(np.int64)
    rowptr = np.concatenate([[0], np.cumsum(cnt)])

    # append self-loops (message y[n] -> dest n), then shard by dest
    loops = np.arange(N, dtype=np.int64)
    row = np.concatenate([row, loops])
    col = np.concatenate([col, loops])

    shard = col // NSHARD
    NG = NSB * NT                        # (superblock, tile) groups
    per_core = []
    counts = np.zeros((NCORES, NG), np.int64)
    for c in range(NCORES):
        m = shard == c
        r = row[m]
        cl = col[m] - c * NSHARD
        pair = r >> 1
        sb = (pair // PAIR_BANK) * 2 + (r & 1)
        g = sb * NT + cl // P
        order = np.argsort(g, kind="stable")
        r, cl, g = r[order], cl[order], g[order]
        counts[c] = np.bincount(g, minlength=NG)
        per_core.append((r, cl, g))

    # 16-granular quotas (idx stream granularity); chunks may then straddle
    # two groups, handled by a second indicator stream on boundary chunks.
    # Each superblock stream is padded to a 128 multiple via its last group.
    quota = (np.ceil(counts.max(axis=0) / 16).astype(np.int64)) * 16   # [NG]
    for s in range(NSB):
        sblen = int(quota[s * NT:(s + 1) * NT].sum())
        quota[(s + 1) * NT - 1] += (-sblen) % P
    qoff = np.concatenate([[0], np.cumsum(quota)])
    total = int(qoff[-1])

    sb_len = [int(quota[s * NT:(s + 1) * NT].sum()) for s in range(NSB)]
    sb_off = np.concatenate([[0], np.cumsum(sb_len)]).astype(np.int64)
    calls = []                            # (sb, stream_start, n_idx)
    for s in range(NSB):
        st = int(sb_off[s])
        while st < int(sb_off[s + 1]):
            n = min(CALL, int(sb_off[s + 1]) - st)
            calls.append((s, st, n))
            st += n

    struct = {"quota": quota.tolist(), "qoff": qoff.tolist(), "total": total,
              "calls": calls}

    # ---- shared arrays ----
    S16, S128 = total // 16, total // 128
    xT = np.zeros((C, N_PAD), np.float32)
    xT[:, :N] = x.T
    xT = np.ascontiguousarray(xT.astype(BF16))
    W_bf = np.ascontiguousarray(np.tile(W, (2, 1)).astype(BF16))  # both halves
    b_bcast = np.ascontiguousarray(np.tile(b[None, :], (P, 1)).astype(np.float32))

    nid = _wrap_node_index()
    valid = nid < N
    rpA = np.zeros((P, NU), np.float32)
    rpB = np.zeros((P, NU), np.float32)
    rpA[valid] = rowptr[nid[valid]]
    rpB[valid] = rowptr[nid[valid] + 1]

    in_maps = []
    for c in range(NCORES):
        r, cl, g = per_core[c]
        cnt_c = counts[c]
        gstart = np.concatenate([[0], np.cumsum(cnt_c)])
        rank = np.arange(len(g)) - gstart[g]
        pos = qoff[g] + rank

        # pad slots must gather *something*; spreading their indices across
        # the bank avoids hammering one 256B HBM row from ~18% of descriptors
        npb = [PAIR_BANK, NPAIR - PAIR_BANK]
        idx_rel = np.concatenate(
            [np.arange(quota[gg]) % npb[gg // NT // 2] for gg in range(NG)])
        colrel = np.full(total, PADCOL, np.float32)     # pads never match iota
        idx_rel[pos] = (r >> 1) - (g // NT // 2) * PAIR_BANK
        colrel[pos] = cl - (g % NT) * P

        # dual indicator streams: slots whose group owns the chunk start go
        # to stream A (fused per-call build); a chunk's second group goes to
        # stream B (small per-boundary-chunk build)
        slot_group = np.repeat(np.arange(NG), quota)
        first_of_chunk = np.repeat(slot_group[::P][:, None], P, 1).ravel()
        a_mask = slot_group == first_of_chunk
        assert np.all((slot_group - first_of_chunk) <= 1), "chunk spans >2 groups"
        colrelA = np.where(a_mask, colrel, PADCOL)
        colrelB = np.where(a_mask, PADCOL, colrel)

        idx16 = np.zeros((16, S16), np.int16)
        idx16[np.arange(total) % 16, np.arange(total) // 16] = idx_rel
        idx16 = np.ascontiguousarray(np.tile(idx16, (8, 1)))

        def pack_colr(cr):
            cc = np.zeros((P, S128), np.float32)
            cc[np.arange(total) % P, np.arange(total) // P] = cr
            return np.ascontiguousarray(cc.astype(BF16))
        colr = pack_colr(colrelA)
        colrB = pack_colr(colrelB)

        pp = np.arange(P)[:, None]
        tt = np.arange(NT)[None, :]
        nd = c * NSHARD + tt * P + pp
        vd = nd < N
        rpdA = np.zeros((P, NT), np.float32)
        rpdB = np.zeros((P, NT), np.float32)
        rpdA[vd] = rowptr[nd[vd]]
        rpdB[vd] = rowptr[nd[vd] + 1]

        in_maps.append({
            "xT": xT, "W": W_bf, "bb": b_bcast, "rpA": rpA, "rpB": rpB,
            "rpdA": np.ascontiguousarray(rpdA),
            "rpdB": np.ascontiguousarray(rpdB),
            "idx16": idx16, "colrel": colr, "colrelB": colrB,
        })
    return in_maps, struct


# ---------------- device program ----------------
def build_program(struct):
    quota = struct["quota"]
    qoff = struct["qoff"]
    total = struct["total"]
    all_calls = struct["calls"]
    S16, S128 = total // 16, total // 128
    phases = os.environ.get("GCN_PHASES", "123")
    skip = os.environ.get("GCN_SKIP", "")
    rep = int(os.environ.get("GCN_REPEAT", "1"))
    maxcalls = int(os.environ.get("GCN_MAXCALLS", "1000000"))

    nc = bacc.Bacc("TRN2", target_bir_lowering=False, debug=True,
                   dynamic_dma_scratch_size=16 * CALL,
                   num_swdge_queues=NQ)
    f32, bf16, i16 = mybir.dt.float32, mybir.dt.bfloat16, mybir.dt.int16

    xT_d = nc.dram_tensor("xT", [C, N_PAD], bf16, kind="ExternalInput")
    W_d = nc.dram_tensor("W", [2 * C, C], bf16, kind="ExternalInput")
    bb_d = nc.dram_tensor("bb", [P, C], f32, kind="ExternalInput")
    rpA_d = nc.dram_tensor("rpA", [P, NU], f32, kind="ExternalInput")
    rpB_d = nc.dram_tensor("rpB", [P, NU], f32, kind="ExternalInput")
    rpdA_d = nc.dram_tensor("rpdA", [P, NT], f32, kind="ExternalInput")
    rpdB_d = nc.dram_tensor("rpdB", [P, NT], f32, kind="ExternalInput")
    idx_d = nc.dram_tensor("idx16", [P, S16], i16, kind="ExternalInput")
    colr_d = nc.dram_tensor("colrel", [P, S128], bf16, kind="ExternalInput")
    colrB_d = nc.dram_tensor("colrelB", [P, S128], bf16, kind="ExternalInput")
    out_d = nc.dram_tensor("out", [P, NT, C], f32, kind="ExternalOutput")
    # y table split at the bank boundary so bank-0 gathers can start while
    # phase 1 is still producing bank-1 rows. y0 = rows [0, 65536] (the odd
    # view of pair 32767 needs row 65536), y1 = rows [65536, N_PAD+2).
    Y0_ROWS = 2 * PAIR_BANK + 2
    Y1_ROWS = N_PAD + 2 - 2 * PAIR_BANK
    y0_d = nc.dram_tensor("ytab0", [Y0_ROWS, C], bf16, kind="Internal")
    y1_d = nc.dram_tensor("ytab1", [Y1_ROWS, C], bf16, kind="Internal")

    with tile.TileContext(nc) as tc:
        with ExitStack() as ctx:
            const = ctx.enter_context(tc.tile_pool(name="const", bufs=1))
            psum_pool = ctx.enter_context(
                tc.tile_pool(name="psum", bufs=4, space="PSUM"))
            psum_mm = ctx.enter_context(
                tc.tile_pool(name="psummm", bufs=4, space="PSUM"))
            dtmp = ctx.enter_context(tc.tile_pool(name="dtmp", bufs=1))
            xtp = ctx.enter_context(tc.tile_pool(name="xt", bufs=2))
            ysbp = ctx.enter_context(tc.tile_pool(name="ysb", bufs=3))
            dvp = ctx.enter_context(tc.tile_pool(name="dvp", bufs=4))
            gbp = ctx.enter_context(tc.tile_pool(name="gb", bufs=8))
            indp = ctx.enter_context(tc.tile_pool(name="ind", bufs=6))
            crp = ctx.enter_context(tc.tile_pool(name="crep", bufs=6))
            indBp = ctx.enter_context(tc.tile_pool(name="indB", bufs=4))
            crpB = ctx.enter_context(tc.tile_pool(name="crB", bufs=4))

            nc.gpsimd.load_library(library_config.mlp)

            W_sb = const.tile([2 * C, C], bf16, tag="W")
            bb_sb = const.tile([P, C], f32, tag="bb")
            iota_i = const.tile([P, P], i16, tag="iota_i")
            iota8 = const.tile([P, CALL // P, P], bf16, tag="iota8")
            dinv_g = const.tile([P, NU], f32, tag="dinv_g")
            dinv_d = const.tile([P, NT], f32, tag="dinv_d")
            acc = const.tile([P, NT * C], f32, tag="acc")
            idx_sb = const.tile([P, S16], i16, tag="idx")
            colr_sb = const.tile([P, S128], bf16, tag="colr")
            colrB_sb = const.tile([P, S128], bf16, tag="colrB")

            nc.sync.dma_start(W_sb[:], W_d[:])
            nc.sync.dma_start(bb_sb[:], bb_d[:])
            nc.sync.dma_start(idx_sb[:], idx_d[:])
            nc.sync.dma_start(colr_sb[:], colr_d[:])
            nc.sync.dma_start(colrB_sb[:], colrB_d[:])
            nc.gpsimd.iota(iota_i[:], pattern=[[1, P]], channel_multiplier=0)
            nc.vector.memset(acc[:], 0.0)
            for j in range(CALL // P):
                nc.vector.tensor_copy(iota8[:, j, :], iota_i[:])
            # zero the tail rows of each y table that fall inside a gather
            # view's declared region but are never written by phase 1
            ztail = const.tile([2, C], bf16, tag="ztail")
            nc.vector.memset(ztail[:], 0.0)
            nc.sync.dma_start(
                bass.AP(y1_d, (Y1_ROWS - 2) * C, [[C, 2], [1, C]]), ztail[:])

            def emit_body():
                # ---- dinv = sqrt(1 / (rowptr[n+1]-rowptr[n]+1)) ----
                for (ad, bd, w, dst) in ((rpA_d, rpB_d, NU, dinv_g),
                                         (rpdA_d, rpdB_d, NT, dinv_d)):
                    ta = dtmp.tile([P, NU], f32, tag="ta", name="ta")
                    tb = dtmp.tile([P, NU], f32, tag="tb", name="tb")
                    nc.sync.dma_start(ta[:, :w], ad[:])
                    nc.sync.dma_start(tb[:, :w], bd[:])
                    nc.vector.tensor_tensor(tb[:, :w], tb[:, :w], ta[:, :w],
                                            mybir.AluOpType.subtract)
                    nc.vector.tensor_scalar_add(tb[:, :w], tb[:, :w], 1.0)
                    nc.vector.reciprocal(ta[:, :w], tb[:, :w])
                    nc.scalar.activation(dst[:], ta[:, :w],
                                         mybir.ActivationFunctionType.Sqrt)

                # ---- phase 1: y = dinv * (x @ W), bf16 table ----
                blocks = []
                base = 0
                while base < N_PAD and "1" in phases:
                    nblk = min(XT_BLOCK, N_PAD - base)
                    blocks.append((base, nblk))
                    base += nblk
                GRP = 4          # windows per ytab write DMA (4096 rows)
                for (base, nblk) in blocks:
                    half = nblk // 2
                    xt = xtp.tile([P, XT_BLOCK // 2], bf16, tag="xt", name="xt")
                    src = bass.AP(xT_d, base,
                                  [[half, 2], [N_PAD, C], [1, half]])
                    nc.sync.dma_start(xt[:, :half], src)
                    nwin = nblk // WIN
                    for wg in range(cdiv(nwin, GRP)):
                        gcnt = min(GRP, nwin - wg * GRP)
                        gbase = base + wg * GRP * WIN
                        ysb = ysbp.tile([P, GRP, WRAP, C], bf16, tag="ysb",
                                        name="ysb")
                        for wi in range(gcnt):
                            w = wg * GRP + wi
                            wbase = base + w * WIN
                            h = (w * WIN) // half
                            foff = (w * WIN) % half
                            ps = psum_pool.tile([P, WRAP * C], f32, tag="mm",
                                                name="mmps")
                            u0 = (wbase // WIN) * WRAP
                            for s in range(WRAP):
                                lhsT = xt[h * C:(h + 1) * C,
                                          foff + s:
                                          foff + s + WRAP * (P - 1) + 1: WRAP]
                                nc.tensor.matmul(ps[:, s * C:(s + 1) * C],
                                                 lhsT,
                                                 W_sb[h * C:(h + 1) * C, :],
                                                 start=True, stop=True)
                            dv = dinv_g[:, u0:u0 + WRAP].unsqueeze(2)
                            dvrep = dvp.tile([P, WRAP, C], f32, tag="dvrep",
                                             name="dvrep")
                            nc.scalar.activation(
                                dvrep[:], dv.broadcast_to((P, WRAP, C)),
                                mybir.ActivationFunctionType.Copy)
                            nc.vector.tensor_tensor(
                                ysb[:, wi, :, :],
                                ps[:].rearrange("p (s c) -> p s c", c=C),
                                dvrep[:], mybir.AluOpType.mult)
                        ap4 = [[WRAP * C, P], [WIN * C, gcnt], [C, WRAP],
                               [1, C]]
                        if gbase + gcnt * WIN <= 2 * PAIR_BANK:
                            dst = bass.AP(y0_d, gbase * C, ap4)
                        else:
                            dst = bass.AP(y1_d, (gbase - 2 * PAIR_BANK) * C,
                                          ap4)
                        # ACT's HWDGE ring: parallel to SP's xt loads
                        nc.scalar.dma_start(dst, ysb[:, :gcnt, :, :])
                        if gbase == 2 * PAIR_BANK:
                            # row 65536 = (p=0, wi=0, s=0) of this group also
                            # belongs to y0 (odd view of pair 32767)
                            nc.scalar.dma_start(
                                bass.AP(y0_d, 2 * PAIR_BANK * C, [[C, 1],
                                                                  [1, C]]),
                                ysb[0:1, 0, 0, :])

                # ---- phase 2: pair-gather + indicator matmuls ----
                calls = all_calls if "2" in phases else []
                calls = calls[:maxcalls]
                grp_first_sb = [None] * NT
                grp_last_sb = [None] * NT
                for t in range(NT):
                    for s in range(NSB):
                        if quota[s * NT + t] > 0:
                            if grp_first_sb[t] is None:
                                grp_first_sb[t] = s
                            grp_last_sb[t] = s

                npair_bank = [PAIR_BANK, NPAIR - PAIR_BANK]
                ytabs = [y0_d, y1_d]
                psum_by_tile = {}
                for ci, (sb, cstart, cn) in enumerate(calls):
                    bk, par = sb // 2, sb % 2
                    nslots = cn // P
                    gbuf = gbp.tile([P, CALL // P, 2 * C], bf16, tag="gbuf",
                                    name="gbuf")
                    view = bass.AP(ytabs[bk], par * C,
                                   [[2 * C, npair_bank[bk]], [1, 2 * C]])
                    if "g" not in skip:
                        nc.gpsimd.dma_gather(
                            gbuf[:, :nslots, :], view,
                            idx_sb[:, cstart // 16: (cstart + cn) // 16],
                            cn, cn, 2 * C, queue_num=ci % NQ)
                    else:
                        nc.vector.memset(gbuf[:, :nslots, :], 0.5)

                    if "m" in skip:     # pure-gather ablation
                        continue
                    # colrep materialized on ACT (idle engine; never contends
                    # with GPSIMD SBUF ports), then a two-stream is_equal on
                    # DVE (tensor_tensor stays in 1-port mode)
                    ind = indp.tile([P, CALL // P, P], bf16, tag="ind",
                                    name="ind")
                    crep = crp.tile([P, CALL // P, P], bf16, tag="crep",
                                    name="crep")
                    if "i" not in skip:
                        cb = colr_sb[:, cstart // P: cstart // P + nslots]
                        nc.scalar.activation(
                            crep[:, :nslots, :],
                            cb.unsqueeze(2).broadcast_to((P, nslots, P)),
                            mybir.ActivationFunctionType.Copy)
                        nc.vector.tensor_tensor(
                            ind[:, :nslots, :], iota8[:, :nslots, :],
                            crep[:, :nslots, :], mybir.AluOpType.is_equal)
                    else:
                        nc.scalar.activation(ind[:, :nslots, :],
                                             iota8[:, :nslots, :],
                                             mybir.ActivationFunctionType.Copy)

                    # second-group indicators for chunks straddling a group
                    # boundary (16-granular quotas)
                    indB_tiles = {}
                    if "i" not in skip:
                        for t in range(NT):
                            g = sb * NT + t
                            g0 = qoff[g]
                            if quota[g] == 0 or g0 % P == 0:
                                continue
                            if not (cstart <= (g0 // P) * P < cstart + cn):
                                continue
                            ch = g0 // P
                            crB = crpB.tile([P, P], bf16, tag="crB",
                                            name="crB")
                            bt = indBp.tile([P, P], bf16, tag="indB",
                                            name="indB")
                            nc.scalar.activation(
                                crB[:],
                                colrB_sb[:, ch:ch + 1].broadcast_to((P, P)),
                                mybir.ActivationFunctionType.Copy)
                            nc.vector.tensor_tensor(
                                bt[:], iota8[:, 0, :], crB[:],
                                mybir.AluOpType.is_equal)
                            indB_tiles[ch] = bt

                    for t in range(NT):
                        g = sb * NT + t
                        q = quota[g]
                        if q == 0:
                            continue
                        g0, g1 = qoff[g], qoff[g] + q
                        lo, hi = max(g0, cstart), min(g1, cstart + cn)
                        if lo >= hi:
                            continue
                        if lo == g0:
                            psum_by_tile[t] = psum_mm.tile(
                                [P, C], f32, tag="mm2", name=f"pst_s{sb}_t{t}")
                        ps = psum_by_tile[t]
                        first_cg, last_cg = g0 // P, (g1 - 1) // P
                        for chunk in range(lo // P, (hi + P - 1) // P):
                            slot = chunk - cstart // P
                            if g0 > chunk * P and chunk in indB_tiles:
                                lhs = indB_tiles[chunk][:]
                            else:
                                lhs = ind[:, slot, :]
                            nc.tensor.matmul(
                                ps[:], lhs, gbuf[:, slot, :C],
                                start=(chunk == first_cg),
                                stop=(chunk == last_cg))
                        if hi == g1:
                            a = acc[:, t * C:(t + 1) * C]
                            if sb == grp_first_sb[t]:
                                # ACT copy: a DVE tensor_copy here would grab
                                # the shared SBUF port pair and stall SWDGE
                                # descriptor generation mid-gather
                                nc.scalar.activation(
                                    a, ps[:],
                                    mybir.ActivationFunctionType.Copy)
                            else:
                                nc.vector.tensor_tensor(
                                    a, a, ps[:], mybir.AluOpType.add)
                            if sb == grp_last_sb[t] and "3" in phases:
                                # fold phase 3 into the final close so the
                                # tail after the last gather stays short;
                                # ACT scale + DVE add never contend with
                                # SWDGE descriptor generation
                                nc.scalar.activation(
                                    a, a, mybir.ActivationFunctionType.Copy,
                                    scale=dinv_d[:, t: t + 1])
                                nc.vector.tensor_tensor(
                                    a, a, bb_sb[:], mybir.AluOpType.add)
                            del psum_by_tile[t]

                for t, ps in list(psum_by_tile.items()):
                    # truncated-call debug runs leave open groups; close them
                    nc.scalar.activation(acc[:, t * C:(t + 1) * C], ps[:],
                                         mybir.ActivationFunctionType.Copy)
                    del psum_by_tile[t]

                # ---- phase 3 is folded into each tile's final close ----
                if "2" not in phases and "3" in phases:
                    for t in range(NT):
                        a = acc[:, t * C:(t + 1) * C]
                        nc.vector.tensor_scalar_mul(a, a, dinv_d[:, t: t + 1])
                        nc.vector.tensor_tensor(a, a, bb_sb[:],
                                                mybir.AluOpType.add)
                nc.sync.dma_start(
                    out_d[:], acc[:].rearrange("p (t c) -> p t c", c=C))

            if rep > 1:
                with tc.For_i(0, rep, 1):
                    emit_body()
            else:
                emit_body()

    nc.compile()
    return nc


# ---------------- entry point ----------------
_CACHE = {}


def kernel(x, edge_index, W, b):
    in_maps, struct = preprocess(x, edge_index, W, b)
    key = (struct["total"], tuple(struct["quota"]))
    if key not in _CACHE:
        _CACHE.clear()
        _CACHE[key] = build_program(struct)
    nc = _CACHE[key]
    res = run_bass_kernel_spmd(nc, in_maps, core_ids=list(range(NCORES)))
    outs = []
    for c in range(NCORES):
        o = res.results[c]["out"]                      # [P, NT, C]
        o = np.transpose(o, (1, 0, 2)).reshape(NT * P, C)[:NSHARD]
        outs.append(o)
    return np.concatenate(outs, axis=0).astype(np.float32)
